# revision 15
# baseline (speedup 1.0000x reference)
"""Trainium2 Bass kernel for nn_MultiHeadAttention_53541062312292.

Reference computation (fp32, NO causal mask, NO 1/sqrt(d) scaling):
    query = q @ Wq.T + bq ; key = q @ Wk.T + bk ; value = q @ Wv.T + bv
    hk = concat(cache_key, key) ; hv = concat(cache_value, value)
    out = softmax(hq @ hk^T) @ hv
    returns (out, hk, hv)

Sharding across 8 NeuronCores: 2 (batch) x 4 (head-groups of 4 heads).
Each core owns 512 rows of Wq/Wk/Wv (its 4 heads), computes the QKV
projections for its batch, and runs full unmasked attention for its
(batch, 4-head) slice over cache+new keys/values.

On-device layout strategy (all matmuls in fp32r at full PE rate):
  - scores are computed transposed: S^T[k, q] = K^T_tile.T @ hq^T so that
    the exp'd probabilities P^T[k, q] are directly usable as the moving
    operand of the AV matmul (contraction over k = partition dim).
  - softmax skips the max-subtraction (scores are within +-50, exp fits
    fp32 comfortably), rowsum = ones.T @ P^T via a K=1-output PE matmul
    accumulated over k-tiles, reciprocal on DVE, broadcast across
    partitions via a K=1 PE matmul, final scale on DVE during PSUM
    eviction.
  - biases are folded into the projection matmuls as a K=1 accumulation
    (ones x bias row).
The cache passthrough part of the hk/hv outputs is pure data movement and
is assembled host-side during the gather step, as is the final
[head, d, q] -> [q, head*d] transpose of the per-core outputs.
"""

import os
import sys
from contextlib import ExitStack

for _p in ("/opt/trn_rl_repo", "/root/.axon_site/_ro/trn_rl_repo"):
    if os.path.isdir(_p) and _p not in sys.path:
        sys.path.insert(0, _p)

import numpy as np

import concourse.bass as bass
from concourse import bacc
import concourse.mybir as mybir
import concourse.tile as tile
from concourse.bass_utils import run_bass_kernel_spmd

f32 = mybir.dt.float32
f32r = mybir.dt.float32r
FT = mybir.ActivationFunctionType

B, Q, E, H, HD, TC = 2, 1024, 2048, 16, 128, 3072
NCORES = 8
GROUPS = 4           # head-groups (tensor parallel dimension)
NH = H // GROUPS     # heads per core
O = NH * HD          # projection output dims per core


def build(T=Q, Eb=E, KC=TC, nh=NH, reps=1):
    """Build the per-core Bass program (SPMD: same program, 8 cores).

    reps>1 wraps the whole computation in an on-device For_i loop — used
    only for timing (wall-clock slope between rep counts).
    """
    assert T % 512 == 0 and Eb % 128 == 0 and KC % 128 == 0
    ET, TGn, TTn, KCT = Eb // 128, T // 512, T // 128, KC // 128
    KNT = KCT + TTn
    Oc = nh * 128

    nc = bacc.Bacc(None, target_bir_lowering=False)
    xT_d = nc.declare_dram_parameter("xT", [Eb, T], f32r, isOutput=False)
    wqT_d = nc.declare_dram_parameter("wqT", [Eb, Oc], f32r, isOutput=False)
    wkT_d = nc.declare_dram_parameter("wkT", [Eb, Oc], f32r, isOutput=False)
    wvT_d = nc.declare_dram_parameter("wvT", [Eb, Oc], f32r, isOutput=False)
    bq_d = nc.declare_dram_parameter("bq", [Oc], f32r, isOutput=False)
    bk_d = nc.declare_dram_parameter("bk", [Oc], f32r, isOutput=False)
    bv_d = nc.declare_dram_parameter("bv", [Oc], f32r, isOutput=False)
    ckT_d = nc.declare_dram_parameter("ckT", [nh, 128, KC], f32r, isOutput=False)
    cv_d = nc.declare_dram_parameter("cv", [nh, KC, 128], f32r, isOutput=False)
    ones_d = nc.declare_dram_parameter("ones", [512], f32r, isOutput=False)
    outT_d = nc.declare_dram_parameter("outT", [nh, 128, T], f32, isOutput=True)
    hkT_d = nc.declare_dram_parameter("hkT", [nh, 128, T], f32, isOutput=True)
    hv_d = nc.declare_dram_parameter("hv", [T, Oc], f32, isOutput=True)

    with ExitStack() as ctx:
        tc = ctx.enter_context(tile.TileContext(nc))
        consts = ctx.enter_context(tc.tile_pool(name="consts", bufs=1))
        proj_sb = ctx.enter_context(tc.tile_pool(name="proj", bufs=1))
        ckpool = ctx.enter_context(tc.tile_pool(name="ck", bufs=2))
        cvpool = ctx.enter_context(tc.tile_pool(name="cvp", bufs=2))
        # one unified PSUM pool: 8 slots of one bank each
        psp = ctx.enter_context(
            tc.tile_pool(name="ps", bufs=8, space=bass.MemorySpace.PSUM)
        )

        ones_row = consts.tile([1, 512], f32r, tag="c1")
        nc.sync.dma_start(ones_row[:], ones_d[:].unsqueeze(0))
        ones_col = consts.tile([128, 1], f32r, tag="c2")
        nc.sync.dma_start(ones_col[:], ones_d[0:128].unsqueeze(1))
        ones_bc = consts.tile([1, 128], f32r, tag="c3")
        nc.sync.dma_start(ones_bc[:], ones_d[0:128].unsqueeze(0))
        btil = {}
        for nm, bd in (("bq", bq_d), ("bk", bk_d), ("bv", bv_d)):
            t_ = consts.tile([1, Oc], f32r, tag=nm)
            nc.sync.dma_start(t_[:], bd[:].unsqueeze(0))
            btil[nm] = t_

        # projection outputs, kept resident for the attention phase
        hqT = proj_sb.tile([128, nh, T], f32r, tag="hqT")    # per head: hq^T [d, q]
        hkTn = proj_sb.tile([128, nh, T], f32r, tag="hkTn")  # per head: knew^T [d, t]
        vnew = proj_sb.tile([128, TTn, Oc], f32r, tag="vnew")  # [t-tile][t, (h d)]

        if reps > 1:
            ctx.enter_context(tc.For_i(0, reps, 1))

        def load_cache_head(h):
            ck_sb = ckpool.tile([128, KC], f32r, tag="ck", name=f"ck{h}")
            nc.sync.dma_start(ck_sb[:], ckT_d[h])
            cv_sb = cvpool.tile([128, KCT, 128], f32r, tag="cv", name=f"cv{h}")
            nc.sync.dma_start(
                cv_sb[:], cv_d[h].rearrange("(kt p) d -> p kt d", p=128)
            )
            return ck_sb, cv_sb

        with (
            tc.tile_pool(name="xp", bufs=1) as xpool,
            tc.tile_pool(name="w", bufs=2) as wpool,
            tc.tile_pool(name="wv", bufs=2) as wvpool,
        ):
            # DMA issue order sets arrival order on the queue: first Q-proj
            # weights for o-tile 0, then x^T in chunks (matmuls can start as
            # soon as the first chunk + w0 land), then head-0 caches.
            w_first = wpool.tile([128, ET, 128], f32r, tag="w", name="w_first")
            wh = max(ET // 2, 1)
            for i in range(0, ET, wh):
                nc.sync.dma_start(
                    w_first[:, i : i + wh, :],
                    wqT_d[i * 128 : (i + wh) * 128, 0:128].rearrange(
                        "(et p) o -> p et o", p=128
                    ),
                )
            xT_sb = xpool.tile([128, ET, T], f32r)
            xc = 2 if ET % 2 == 0 else 1
            for i in range(0, ET, xc):
                nc.sync.dma_start(
                    xT_sb[:, i : i + xc, :],
                    xT_d[i * 128 : (i + xc) * 128, :].rearrange(
                        "(et p) t -> p et t", p=128
                    ),
                )

            # ---- Q and K projections: Y^T[o, t] = W^T.T @ x^T (+bias) ----
            for dest, w_d, bname, store_k in (
                (hqT, wqT_d, "bq", False),
                (hkTn, wkT_d, "bk", True),
            ):
                for ot in range(nh):
                    if bname == "bq" and ot == 0:
                        w_sb = w_first
                    else:
                        w_sb = wpool.tile([128, ET, 128], f32r, tag="w",
                                          name=f"w{bname}{ot}")
                        nc.sync.dma_start(
                            w_sb[:],
                            w_d[:, ot * 128 : (ot + 1) * 128].rearrange(
                                "(et p) o -> p et o", p=128
                            ),
                        )
                    for tg in range(TGn):
                        ts = slice(tg * 512, (tg + 1) * 512)
                        ps = psp.tile([128, 512], f32, tag="s", name="psqk")
                        nc.tensor.matmul(
                            ps[:],
                            btil[bname][:, ot * 128 : (ot + 1) * 128],
                            ones_row[:],
                            start=True,
                            stop=False,
                        )
                        for et in range(ET):
                            nc.tensor.matmul(
                                ps[:],
                                w_sb[:, et, :],
                                xT_sb[:, et, ts],
                                start=False,
                                stop=(et == ET - 1),
                            )
                        nc.vector.tensor_copy(dest[:, ot, ts], ps[:])
                    if store_k:
                        nc.sync.dma_start(hkT_d[ot], dest[:, ot, :].bitcast(f32))

            # head-0 caches: issued after the projection weights so they don't
            # head-of-line-block the weight stream (needed much later)
            cache0 = load_cache_head(0)

            # ---- V projection: Yv[t, o] = x^T.T @ Wv^T (+bias) ----
            # et-outer with streamed weight tiles; all TTn psum banks live.
            psv = [
                psp.tile([128, Oc], f32, tag="s", name=f"psv{tt}")
                for tt in range(TTn)
            ]
            for tt in range(TTn):
                nc.tensor.matmul(
                    psv[tt][:], ones_row[:, 0:128], btil["bv"][:],
                    start=True, stop=False,
                )
            for et in range(ET):
                wv_sb = wvpool.tile([128, Oc], f32r, tag="wv", name=f"wv{et}")
                nc.sync.dma_start(wv_sb[:], wvT_d[et * 128 : (et + 1) * 128, :])
                for tt in range(TTn):
                    nc.tensor.matmul(
                        psv[tt][:],
                        xT_sb[:, et, tt * 128 : (tt + 1) * 128],
                        wv_sb[:],
                        start=False,
                        stop=(et == ET - 1),
                    )
            for tt in range(TTn):
                nc.vector.tensor_copy(vnew[:, tt, :], psv[tt][:])
                nc.sync.dma_start(
                    hv_d[tt * 128 : (tt + 1) * 128, :], vnew[:, tt, :].bitcast(f32)
                )

        # ---- attention, one head at a time, software-pipelined over k ----
        with (
            tc.tile_pool(name="pT", bufs=6) as pTpool,
            tc.tile_pool(name="att", bufs=2) as attpool,
            tc.tile_pool(name="sm", bufs=2) as smpool,
        ):
            cache = cache0
            for h in range(nh):
                ck_sb, cv_sb = cache
                if h + 1 < nh:
                    cache = load_cache_head(h + 1)

                def kv(kb):
                    if kb < KCT:
                        return ck_sb[:, kb * 128 : (kb + 1) * 128], cv_sb[:, kb, :]
                    j = kb - KCT
                    return (hkTn[:, h, j * 128 : (j + 1) * 128],
                            vnew[:, j, h * 128 : (h + 1) * 128])

                oacc = [
                    psp.tile([128, 512], f32, tag="s", name=f"oacc{tg}")
                    for tg in range(TGn)
                ]
                rs = [
                    psp.tile([1, 512], f32, tag="s", name=f"rs{tg}")[:]
                    for tg in range(TGn)
                ]
                # one-stage software pipeline: scores/exp for kb, AV for kb-1
                pT_prev = None
                for kb in range(KNT + 1):
                    pT_cur = []
                    if kb < KNT:
                        kt_ap, _ = kv(kb)
                        for tg in range(TGn):
                            ts = slice(tg * 512, (tg + 1) * 512)
                            s_ps = psp.tile([128, 512], f32, tag="s",
                                            name=f"s{kb}_{tg}")
                            nc.tensor.matmul(
                                s_ps[:], kt_ap, hqT[:, h, ts],
                                start=True, stop=True,
                            )
                            pT = pTpool.tile([128, 512], f32r, tag="pT",
                                             name=f"pT{kb}_{tg}")
                            nc.scalar.activation(pT[:], s_ps[:], FT.Exp)
                            pT_cur.append(pT)
                    if kb > 0:
                        _, v_ap = kv(kb - 1)
                        first, last = (kb - 1 == 0), (kb - 1 == KNT - 1)
                        for tg in range(TGn):
                            nc.tensor.matmul(
                                oacc[tg][:], v_ap, pT_prev[tg][:],
                                start=first, stop=last,
                            )
                            nc.tensor.matmul(
                                rs[tg], ones_col[:], pT_prev[tg][:],
                                start=first, stop=last,
                            )
                    pT_prev = pT_cur

                # normalize: out^T[d, q] = oacc[d, q] * (1 / rowsum[q])
                recip = smpool.tile([1, T], f32r, tag="recip")
                with nc.allow_low_precision(reason="f32r recip feeds PE broadcast"):
                    for tg in range(TGn):
                        nc.vector.reciprocal(
                            recip[:, tg * 512 : (tg + 1) * 512], rs[tg]
                        )
                bc_ps = [
                    psp.tile([128, 512], f32, tag="s", name=f"bc{tg}")
                    for tg in range(TGn)
                ]
                for tg in range(TGn):
                    ts = slice(tg * 512, (tg + 1) * 512)
                    nc.tensor.matmul(
                        bc_ps[tg][:], ones_bc[:], recip[:, ts],
                        start=True, stop=True,
                    )
                bc_sb = smpool.tile([128, T], f32, tag="bc")
                out_sb = attpool.tile([128, T], f32, tag="out")
                for tg in range(TGn):
                    ts = slice(tg * 512, (tg + 1) * 512)
                    nc.vector.tensor_copy(bc_sb[:, ts], bc_ps[tg][:])
                    nc.vector.tensor_mul(out_sb[:, ts], oacc[tg][:], bc_sb[:, ts])
                nc.sync.dma_start(outT_d[h], out_sb[:])
    nc.finalize()
    return nc


def shard_inputs(q, cache_key, cache_value, Wq, bq, Wk, bk, Wv, bv):
    """Host-side shard + relayout of the full inputs into 8 per-core maps."""
    in_maps = []
    for c in range(NCORES):
        b, g = divmod(c, GROUPS)
        rows = slice(g * O, (g + 1) * O)
        heads = slice(g * NH, (g + 1) * NH)
        in_maps.append(
            {
                "xT": np.ascontiguousarray(q[b].T),
                "wqT": np.ascontiguousarray(Wq[rows].T),
                "wkT": np.ascontiguousarray(Wk[rows].T),
                "wvT": np.ascontiguousarray(Wv[rows].T),
                "bq": np.ascontiguousarray(bq[rows]),
                "bk": np.ascontiguousarray(bk[rows]),
                "bv": np.ascontiguousarray(bv[rows]),
                "ckT": np.ascontiguousarray(cache_key[b][:, heads, :].transpose(1, 2, 0)),
                "cv": np.ascontiguousarray(cache_value[b][:, heads, :].transpose(1, 0, 2)),
                "ones": np.ones(512, np.float32),
            }
        )
    return in_maps


def gather_outputs(results, cache_key, cache_value):
    """Host-side gather: assemble full (out, hk, hv) from 8 per-core maps."""
    out = np.empty((B, Q, E), np.float32)
    hk = np.empty((B, TC + Q, H, HD), np.float32)
    hv = np.empty((B, TC + Q, H, HD), np.float32)
    hk[:, :TC] = cache_key
    hv[:, :TC] = cache_value
    for c in range(NCORES):
        b, g = divmod(c, GROUPS)
        heads = slice(g * NH, (g + 1) * NH)
        rc = results[c]
        out[b, :, g * O : (g + 1) * O] = rc["outT"].reshape(O, Q).T
        hk[b, TC:, heads, :] = rc["hkT"].reshape(O, Q).T.reshape(Q, NH, HD)
        hv[b, TC:, heads, :] = rc["hv"].reshape(Q, NH, HD)
    return out, hk, hv


_NC_CACHE = None


def _get_nc():
    global _NC_CACHE
    if _NC_CACHE is None:
        _NC_CACHE = build()
    return _NC_CACHE


def kernel(q, cache_key, cache_value, Wq, bq, Wk, bk, Wv, bv):
    args = [
        np.ascontiguousarray(np.asarray(a, dtype=np.float32))
        for a in (q, cache_key, cache_value, Wq, bq, Wk, bk, Wv, bv)
    ]
    q, cache_key, cache_value = args[0], args[1], args[2]
    in_maps = shard_inputs(*args)
    res = run_bass_kernel_spmd(_get_nc(), in_maps, list(range(NCORES)))
    return gather_outputs(res.results, cache_key, cache_value)


# revision 19
# speedup vs baseline: 1.0467x; 1.0467x over previous
"""Trainium2 Bass kernel for nn_MultiHeadAttention_53541062312292.

Reference computation (fp32, NO causal mask, NO 1/sqrt(d) scaling):
    query = q @ Wq.T + bq ; key = q @ Wk.T + bk ; value = q @ Wv.T + bv
    hk = concat(cache_key, key) ; hv = concat(cache_value, value)
    out = softmax(hq @ hk^T) @ hv
    returns (out, hk, hv)

Sharding across 8 NeuronCores: 2 (batch) x 4 (head-groups of 4 heads).
Each core owns 512 rows of Wq/Wk/Wv (its 4 heads), computes the QKV
projections for its batch, and runs full unmasked attention for its
(batch, 4-head) slice over cache+new keys/values.

On-device layout strategy (all matmuls in fp32r at full PE rate):
  - scores are computed transposed: S^T[k, q] = K^T_tile.T @ hq^T so that
    the exp'd probabilities P^T[k, q] are directly usable as the moving
    operand of the AV matmul (contraction over k = partition dim).
  - softmax skips the max-subtraction (scores are within +-50, exp fits
    fp32 comfortably), rowsum = ones.T @ P^T via a K=1-output PE matmul
    accumulated over k-tiles, reciprocal on DVE, broadcast across
    partitions via a K=1 PE matmul, final scale on DVE during PSUM
    eviction.
  - biases are folded into the projection matmuls as a K=1 accumulation
    (ones x bias row).
The cache passthrough part of the hk/hv outputs is pure data movement and
is assembled host-side during the gather step, as is the final
[head, d, q] -> [q, head*d] transpose of the per-core outputs.
"""

import os
import sys
from contextlib import ExitStack

for _p in ("/opt/trn_rl_repo", "/root/.axon_site/_ro/trn_rl_repo"):
    if os.path.isdir(_p) and _p not in sys.path:
        sys.path.insert(0, _p)

import numpy as np

import concourse.bass as bass
from concourse import bacc
import concourse.mybir as mybir
import concourse.tile as tile
from concourse.bass_utils import run_bass_kernel_spmd

f32 = mybir.dt.float32
f32r = mybir.dt.float32r
FT = mybir.ActivationFunctionType

B, Q, E, H, HD, TC = 2, 1024, 2048, 16, 128, 3072
NCORES = 8
GROUPS = 4           # head-groups (tensor parallel dimension)
NH = H // GROUPS     # heads per core
O = NH * HD          # projection output dims per core


def build(T=Q, Eb=E, KC=TC, nh=NH, reps=1, phase="all", bfv=False, dmasplit=False):
    """Build the per-core Bass program (SPMD: same program, 8 cores).

    reps>1 wraps the whole computation in an on-device For_i loop — used
    only for timing (wall-clock slope between rep counts).
    """
    assert T % 512 == 0 and Eb % 128 == 0 and KC % 128 == 0
    ET, TGn, TTn, KCT = Eb // 128, T // 512, T // 128, KC // 128
    KNT = KCT + TTn
    Oc = nh * 128

    nc = bacc.Bacc(None, target_bir_lowering=False)
    xT_d = nc.declare_dram_parameter("xT", [Eb, T], f32r, isOutput=False)
    wqT_d = nc.declare_dram_parameter("wqT", [Eb, Oc], f32r, isOutput=False)
    wkT_d = nc.declare_dram_parameter("wkT", [Eb, Oc], f32r, isOutput=False)
    wvT_d = nc.declare_dram_parameter("wvT", [Eb, Oc], f32r, isOutput=False)
    bq_d = nc.declare_dram_parameter("bq", [Oc], f32r, isOutput=False)
    bk_d = nc.declare_dram_parameter("bk", [Oc], f32r, isOutput=False)
    bv_d = nc.declare_dram_parameter("bv", [Oc], f32r, isOutput=False)
    ckT_d = nc.declare_dram_parameter("ckT", [nh, 128, KC], f32r, isOutput=False)
    bf16 = mybir.dt.bfloat16
    vdt = bf16 if bfv else f32r
    cv_d = nc.declare_dram_parameter("cv", [nh, KC, 128], vdt, isOutput=False)
    ones_d = nc.declare_dram_parameter("ones", [512], f32r, isOutput=False)
    outT_d = nc.declare_dram_parameter("outT", [nh, 128, T], f32, isOutput=True)
    hkT_d = nc.declare_dram_parameter("hkT", [nh, 128, T], f32, isOutput=True)
    hv_d = nc.declare_dram_parameter("hv", [T, Oc], f32, isOutput=True)

    with ExitStack() as ctx:
        tc = ctx.enter_context(tile.TileContext(nc))
        consts = ctx.enter_context(tc.tile_pool(name="consts", bufs=1))
        proj_sb = ctx.enter_context(tc.tile_pool(name="proj", bufs=1))
        ckpool = ctx.enter_context(tc.tile_pool(name="ck", bufs=2))
        cvpool = ctx.enter_context(tc.tile_pool(name="cvp", bufs=2))
        # one unified PSUM pool: 8 slots of one bank each
        psp = ctx.enter_context(
            tc.tile_pool(name="ps", bufs=8, space=bass.MemorySpace.PSUM)
        )

        ones_row = consts.tile([1, 512], f32r, tag="c1")
        nc.sync.dma_start(ones_row[:], ones_d[:].unsqueeze(0))
        ones_col = consts.tile([128, 1], f32r, tag="c2")
        nc.sync.dma_start(ones_col[:], ones_d[0:128].unsqueeze(1))
        ones_bc = consts.tile([1, 128], f32r, tag="c3")
        nc.sync.dma_start(ones_bc[:], ones_d[0:128].unsqueeze(0))
        btil = {}
        for nm, bd in (("bq", bq_d), ("bk", bk_d), ("bv", bv_d)):
            t_ = consts.tile([1, Oc], f32r, tag=nm)
            nc.sync.dma_start(t_[:], bd[:].unsqueeze(0))
            btil[nm] = t_

        # projection outputs, kept resident for the attention phase
        hqT = proj_sb.tile([128, nh, T], f32r, tag="hqT")    # per head: hq^T [d, q]
        hkTn = proj_sb.tile([128, nh, T], f32r, tag="hkTn")  # per head: knew^T [d, t]
        vnew = proj_sb.tile([128, TTn, Oc], vdt, tag="vnew")  # [t-tile][t, (h d)]

        if reps > 1:
            ctx.enter_context(tc.For_i(0, reps, 1))

        dma2 = nc.gpsimd.dma_start if dmasplit else nc.sync.dma_start

        def load_cache_head(h):
            ck_sb = ckpool.tile([128, KC], f32r, tag="ck", name=f"ck{h}")
            dma2(ck_sb[:], ckT_d[h])
            cv_sb = cvpool.tile([128, KCT, 128], vdt, tag="cv", name=f"cv{h}")
            dma2(cv_sb[:], cv_d[h].rearrange("(kt p) d -> p kt d", p=128))
            return ck_sb, cv_sb

        with (
            tc.tile_pool(name="xp", bufs=1) as xpool,
            tc.tile_pool(name="w", bufs=2) as wpool,
            tc.tile_pool(name="wv", bufs=2) as wvpool,
        ):
            # DMA issue order sets arrival order on the queue: first Q-proj
            # weights for o-tile 0, then x^T in chunks (matmuls can start as
            # soon as the first chunk + w0 land), then head-0 caches.
            w_first = wpool.tile([128, ET, 128], f32r, tag="w", name="w_first")
            wh = max(ET // 2, 1)
            for i in range(0, ET, wh):
                nc.sync.dma_start(
                    w_first[:, i : i + wh, :],
                    wqT_d[i * 128 : (i + wh) * 128, 0:128].rearrange(
                        "(et p) o -> p et o", p=128
                    ),
                )
            xT_sb = xpool.tile([128, ET, T], f32r)
            xc = 2 if ET % 2 == 0 else 1
            for i in range(0, ET, xc):
                nc.sync.dma_start(
                    xT_sb[:, i : i + xc, :],
                    xT_d[i * 128 : (i + xc) * 128, :].rearrange(
                        "(et p) t -> p et t", p=128
                    ),
                )

            # ---- Q and K projections: Y^T[o, t] = W^T.T @ x^T (+bias) ----
            for dest, w_d, bname, store_k in (
                (hqT, wqT_d, "bq", False),
                (hkTn, wkT_d, "bk", True),
            ):
                for ot in range(nh):
                    if bname == "bq" and ot == 0:
                        w_sb = w_first
                    else:
                        w_sb = wpool.tile([128, ET, 128], f32r, tag="w",
                                          name=f"w{bname}{ot}")
                        nc.sync.dma_start(
                            w_sb[:],
                            w_d[:, ot * 128 : (ot + 1) * 128].rearrange(
                                "(et p) o -> p et o", p=128
                            ),
                        )
                    for tg in range(TGn):
                        ts = slice(tg * 512, (tg + 1) * 512)
                        ps = psp.tile([128, 512], f32, tag="s", name="psqk")
                        nc.tensor.matmul(
                            ps[:],
                            btil[bname][:, ot * 128 : (ot + 1) * 128],
                            ones_row[:],
                            start=True,
                            stop=False,
                        )
                        for et in range(ET):
                            nc.tensor.matmul(
                                ps[:],
                                w_sb[:, et, :],
                                xT_sb[:, et, ts],
                                start=False,
                                stop=(et == ET - 1),
                            )
                        nc.vector.tensor_copy(dest[:, ot, ts], ps[:])
                    if store_k:
                        dma2(hkT_d[ot], dest[:, ot, :].bitcast(f32))

            # head-0 caches: issued after the projection weights so they don't
            # head-of-line-block the weight stream (needed much later)
            cache0 = load_cache_head(0)

            # ---- V projection: Yv[t, o] = x^T.T @ Wv^T (+bias) ----
            # et-outer with streamed weight tiles; all TTn psum banks live.
            psv = [
                psp.tile([128, Oc], f32, tag="s", name=f"psv{tt}")
                for tt in range(TTn)
            ]
            for tt in range(TTn):
                nc.tensor.matmul(
                    psv[tt][:], ones_row[:, 0:128], btil["bv"][:],
                    start=True, stop=False,
                )
            for et in range(ET):
                wv_sb = wvpool.tile([128, Oc], f32r, tag="wv", name=f"wv{et}")
                dma2(wv_sb[:], wvT_d[et * 128 : (et + 1) * 128, :])
                for tt in range(TTn):
                    nc.tensor.matmul(
                        psv[tt][:],
                        xT_sb[:, et, tt * 128 : (tt + 1) * 128],
                        wv_sb[:],
                        start=False,
                        stop=(et == ET - 1),
                    )
            for tt in range(TTn):
                nc.vector.tensor_copy(vnew[:, tt, :], psv[tt][:])
                if bfv:
                    vout = wvpool.tile([128, Oc], f32, tag="vout",
                                       name=f"vout{tt}")
                    nc.scalar.activation(vout[:], psv[tt][:], FT.Copy)
                    dma2(hv_d[tt * 128 : (tt + 1) * 128, :], vout[:])
                else:
                    dma2(
                        hv_d[tt * 128 : (tt + 1) * 128, :],
                        vnew[:, tt, :].bitcast(f32),
                    )

        # ---- attention, one head at a time, software-pipelined over k ----
        with (
            tc.tile_pool(name="pT", bufs=6) as pTpool,
            tc.tile_pool(name="att", bufs=2) as attpool,
            tc.tile_pool(name="sm", bufs=2) as smpool,
        ):
            cache = cache0
            for h in range(nh if phase == "all" else 0):
                ck_sb, cv_sb = cache
                if h + 1 < nh:
                    cache = load_cache_head(h + 1)

                def kv(kb):
                    if kb < KCT:
                        return ck_sb[:, kb * 128 : (kb + 1) * 128], cv_sb[:, kb, :]
                    j = kb - KCT
                    return (hkTn[:, h, j * 128 : (j + 1) * 128],
                            vnew[:, j, h * 128 : (h + 1) * 128])

                oacc = [
                    psp.tile([128, 512], f32, tag="s", name=f"oacc{tg}")
                    for tg in range(TGn)
                ]
                rs = [
                    psp.tile([1, 512], f32, tag="s", name=f"rs{tg}")[:]
                    for tg in range(TGn)
                ]
                # one-stage software pipeline: scores/exp for kb, AV for kb-1
                pT_prev = None
                for kb in range(KNT + 1):
                    pT_cur = []
                    if kb < KNT:
                        kt_ap, _ = kv(kb)
                        for tg in range(TGn):
                            ts = slice(tg * 512, (tg + 1) * 512)
                            s_ps = psp.tile([128, 512], f32, tag="s",
                                            name=f"s{kb}_{tg}")
                            nc.tensor.matmul(
                                s_ps[:], kt_ap, hqT[:, h, ts],
                                start=True, stop=True,
                            )
                            pT = pTpool.tile([128, 512], f32r, tag="pT",
                                             name=f"pT{kb}_{tg}")
                            nc.scalar.activation(pT[:], s_ps[:], FT.Exp)
                            pT_cur.append(pT)
                    if kb > 0:
                        _, v_ap = kv(kb - 1)
                        first, last = (kb - 1 == 0), (kb - 1 == KNT - 1)
                        for tg in range(TGn):
                            nc.tensor.matmul(
                                oacc[tg][:], v_ap, pT_prev[tg][:],
                                start=first, stop=last,
                            )
                            nc.tensor.matmul(
                                rs[tg], ones_col[:], pT_prev[tg][:],
                                start=first, stop=last,
                            )
                    pT_prev = pT_cur

                # normalize: out^T[d, q] = oacc[d, q] * (1 / rowsum[q])
                recip = smpool.tile([1, T], f32r, tag="recip")
                with nc.allow_low_precision(reason="f32r recip feeds PE broadcast"):
                    for tg in range(TGn):
                        nc.vector.reciprocal(
                            recip[:, tg * 512 : (tg + 1) * 512], rs[tg]
                        )
                bc_ps = [
                    psp.tile([128, 512], f32, tag="s", name=f"bc{tg}")
                    for tg in range(TGn)
                ]
                for tg in range(TGn):
                    ts = slice(tg * 512, (tg + 1) * 512)
                    nc.tensor.matmul(
                        bc_ps[tg][:], ones_bc[:], recip[:, ts],
                        start=True, stop=True,
                    )
                bc_sb = smpool.tile([128, T], f32, tag="bc")
                out_sb = attpool.tile([128, T], f32, tag="out")
                for tg in range(TGn):
                    ts = slice(tg * 512, (tg + 1) * 512)
                    nc.vector.tensor_copy(bc_sb[:, ts], bc_ps[tg][:])
                    nc.vector.tensor_mul(out_sb[:, ts], oacc[tg][:], bc_sb[:, ts])
                dma2(outT_d[h], out_sb[:])
    nc.finalize()
    return nc


def shard_inputs(q, cache_key, cache_value, Wq, bq, Wk, bk, Wv, bv, bfv=False):
    """Host-side shard + relayout of the full inputs into 8 per-core maps."""
    import ml_dtypes
    vdt = ml_dtypes.bfloat16 if bfv else np.float32
    in_maps = []
    for c in range(NCORES):
        b, g = divmod(c, GROUPS)
        rows = slice(g * O, (g + 1) * O)
        heads = slice(g * NH, (g + 1) * NH)
        in_maps.append(
            {
                "xT": np.ascontiguousarray(q[b].T),
                "wqT": np.ascontiguousarray(Wq[rows].T),
                "wkT": np.ascontiguousarray(Wk[rows].T),
                "wvT": np.ascontiguousarray(Wv[rows].T),
                "bq": np.ascontiguousarray(bq[rows]),
                "bk": np.ascontiguousarray(bk[rows]),
                "bv": np.ascontiguousarray(bv[rows]),
                "ckT": np.ascontiguousarray(cache_key[b][:, heads, :].transpose(1, 2, 0)),
                "cv": np.ascontiguousarray(cache_value[b][:, heads, :].transpose(1, 0, 2)).astype(vdt),
                "ones": np.ones(512, np.float32),
            }
        )
    return in_maps


def gather_outputs(results, cache_key, cache_value):
    """Host-side gather: assemble full (out, hk, hv) from 8 per-core maps."""
    out = np.empty((B, Q, E), np.float32)
    hk = np.empty((B, TC + Q, H, HD), np.float32)
    hv = np.empty((B, TC + Q, H, HD), np.float32)
    hk[:, :TC] = cache_key
    hv[:, :TC] = cache_value
    for c in range(NCORES):
        b, g = divmod(c, GROUPS)
        heads = slice(g * NH, (g + 1) * NH)
        rc = results[c]
        out[b, :, g * O : (g + 1) * O] = rc["outT"].reshape(O, Q).T
        hk[b, TC:, heads, :] = rc["hkT"].reshape(O, Q).T.reshape(Q, NH, HD)
        hv[b, TC:, heads, :] = rc["hv"].reshape(Q, NH, HD)
    return out, hk, hv


_NC_CACHE = None


def _get_nc():
    global _NC_CACHE
    if _NC_CACHE is None:
        _NC_CACHE = build()
    return _NC_CACHE


def kernel(q, cache_key, cache_value, Wq, bq, Wk, bk, Wv, bv):
    args = [
        np.ascontiguousarray(np.asarray(a, dtype=np.float32))
        for a in (q, cache_key, cache_value, Wq, bq, Wk, bk, Wv, bv)
    ]
    q, cache_key, cache_value = args[0], args[1], args[2]
    in_maps = shard_inputs(*args)
    res = run_bass_kernel_spmd(_get_nc(), in_maps, list(range(NCORES)))
    return gather_outputs(res.results, cache_key, cache_value)
